# revision 23
# baseline (speedup 1.0000x reference)
"""Trainium2 Bass kernel for nn_MultiHeadAttention (sparse_attention).

Sharding: 8 cores = 2 batches x 4-way sequence split. Core c handles
batch b=c//4 and q-columns r::4 (r=c%4) of that batch -- a perfectly
balanced, SPMD-uniform causal split. Each core computes all 16 heads
for its 512 q positions (QKV projections for full S are replicated
within a batch group), the fc projection fully locally (K=1024), and
only an 8KB AllReduce of LayerNorm statistics crosses cores.

Layout: everything feature-on-partition / sequence-on-free.
  qpT/kpT: (dk, q) bf16;  vp: (s, dk) bf16 with a ones-column appended
  (the ones row of the AV output is the softmax denominator).
Scores are computed transposed (k on partitions, q on free) so softmax
denominators come free from the AV matmul and no transposes are needed
anywhere. Causality is enforced by data-driven multiplicative masks
(host-computed per core) so the instruction stream is identical on all
8 cores. Softmax needs no max-subtraction: scores are (qp.kp)/64 with
|s| < ~0.05, so exp() cannot overflow.
"""

import sys

for _p in ("/opt/trn_rl_repo",):
    if _p not in sys.path:
        sys.path.insert(0, _p)

from contextlib import ExitStack

import ml_dtypes
import numpy as np

import concourse.bacc as bacc
import concourse.tile as tile
from concourse import mybir
from concourse.bass_utils import run_bass_kernel_spmd

BF16 = mybir.dt.bfloat16
F32 = mybir.dt.float32
F32R = mybir.dt.float32r
NPBF16 = ml_dtypes.bfloat16
AF = mybir.ActivationFunctionType

B, S, E, H, DK = 2, 2048, 1024, 16, 64
NPAIR = 8  # head pairs
SQ = 512  # q columns per core
EPS = 1e-4
GROUPS = [[0, 1, 2, 3], [4, 5, 6, 7]]

_NC_CACHE = None
_MASKS = None


def _emit(nc):
    qt = nc.dram_tensor("qt", [128, NPAIR * SQ], BF16, kind="ExternalInput")
    kt = nc.dram_tensor("kt", [128, NPAIR * S], BF16, kind="ExternalInput")
    vt = nc.dram_tensor("vt", [128, NPAIR * S], BF16, kind="ExternalInput")
    wq = nc.dram_tensor("wq", [128, NPAIR * 128], BF16, kind="ExternalInput")
    wk = nc.dram_tensor("wk", [128, NPAIR * 128], BF16, kind="ExternalInput")
    wv = nc.dram_tensor("wv", [128, NPAIR * 128], BF16, kind="ExternalInput")
    bq = nc.dram_tensor("bq", [128, NPAIR], F32, kind="ExternalInput")
    bk = nc.dram_tensor("bk", [128, NPAIR], F32, kind="ExternalInput")
    bv = nc.dram_tensor("bv", [128, NPAIR], F32, kind="ExternalInput")
    wfc = nc.dram_tensor("wfc", [128, 8 * E], F32R, kind="ExternalInput")
    vres = nc.dram_tensor("vres", [4, 128, E], F32, kind="ExternalInput")
    gamma = nc.dram_tensor("gamma", [128, 4], F32, kind="ExternalInput")
    beta = nc.dram_tensor("beta", [128, 4], F32, kind="ExternalInput")
    maskin = nc.dram_tensor("mask", [128, 16 * 64], BF16, kind="ExternalInput")
    out = nc.dram_tensor("out", [4, 128, E], F32, kind="ExternalOutput")

    ones_col_c = nc.inline_tensor(np.ones((128, 1), np.float32), "ones_col")
    ones_row_c = nc.inline_tensor(np.ones((1, 128), np.float32), "ones_row")
    blk2_np = np.zeros((33, 128), np.float32)
    blk2_np[0, :64] = 1.0
    blk2_np[32, 64:] = 1.0
    blk2_c = nc.inline_tensor(blk2_np.astype(ml_dtypes.bfloat16), "blk2")
    eps_c = nc.inline_tensor(np.full((1, 1), EPS, np.float32), "epsc")
    eps_b_c = nc.inline_tensor(np.full((128, 1), EPS, np.float32), "epsbc")

    with tile.TileContext(nc) as tc, ExitStack() as ex:
        cst = ex.enter_context(tc.tile_pool(name="cst", bufs=1))
        ones_col_sb = cst.tile([128, 1], F32)
        ones_row_sb = cst.tile([1, 128], F32)
        blk2_sb = cst.tile([33, 128], BF16)
        bq_sb = cst.tile([128, NPAIR], F32)
        bk_sb = cst.tile([128, NPAIR], F32)
        bv_sb = cst.tile([128, NPAIR], F32)
        eps_sb = cst.tile([1, 1], F32)
        eps_sb_b = cst.tile([128, 1], F32)
        nc.sync.dma_start(out=ones_col_sb[:], in_=ones_col_c.ap())
        nc.sync.dma_start(out=ones_row_sb[:], in_=ones_row_c.ap())
        nc.sync.dma_start(out=blk2_sb[:], in_=blk2_c.ap())
        nc.sync.dma_start(out=eps_sb[:], in_=eps_c.ap())
        nc.sync.dma_start(out=eps_sb_b[:], in_=eps_b_c.ap())
        nc.sync.dma_start(out=bq_sb[:], in_=bq.ap())
        nc.sync.dma_start(out=bk_sb[:], in_=bk.ap())
        nc.sync.dma_start(out=bv_sb[:], in_=bv.ap())

        dramw = ex.enter_context(tc.tile_pool(name="dramw", bufs=1, space="DRAM"))
        warm_in = dramw.tile([1, 16], F32)
        warm_out = dramw.tile([1, 16], F32)
        warm_sb = ex.enter_context(tc.tile_pool(name="warmp", bufs=1)).tile([1, 16], F32)
        nc.vector.memset(warm_sb[:], 0.0)
        nc.sync.dma_start(out=warm_in[:], in_=warm_sb[:])
        nc.gpsimd.collective_compute(
            "AllReduce",
            mybir.AluOpType.add,
            replica_groups=GROUPS,
            ins=[warm_in.opt()],
            outs=[warm_out.opt()],
        )
        # live through phase 3
        poolC = ex.enter_context(tc.tile_pool(name="poolC", bufs=1))
        OT = poolC.tile([128, NPAIR * SQ], F32R)
        wfc_sb = poolC.tile([128, 8 * E], F32R)
        nc.sync.dma_start(out=wfc_sb[:], in_=wfc.ap())
        # live through phase 2
        exA = ex.enter_context(ExitStack())
        poolA = exA.enter_context(tc.tile_pool(name="poolA", bufs=1))
        qpT0 = poolA.tile([128, NPAIR * SQ], BF16)
        qpT1 = poolA.tile([128, NPAIR * SQ], BF16)
        kpT = poolA.tile([128, NPAIR * S], BF16)
        vp_all = poolA.tile([128, H * 16 * 65], BF16)

        # ---------------- phase 1: load + projections ----------------
        with ExitStack() as ex1:
            p1 = ex1.enter_context(tc.tile_pool(name="p1", bufs=1))
            qt_sb = p1.tile([128, NPAIR * SQ], BF16)
            kt_sb = p1.tile([128, NPAIR * S], BF16)
            vt_sb = p1.tile([128, NPAIR * S], BF16)
            wq_sb = p1.tile([128, NPAIR * 128], BF16)
            wk_sb = p1.tile([128, NPAIR * 128], BF16)
            wv_sb = p1.tile([128, NPAIR * 128], BF16)
            ps1 = ex1.enter_context(tc.tile_pool(name="ps1", bufs=6, space="PSUM"))

            for w_in, w_sb in ((wq, wq_sb), (wk, wk_sb), (wv, wv_sb)):
                nc.sync.dma_start(out=w_sb[:], in_=w_in.ap())
            for p in range(NPAIR):
                nc.sync.dma_start(
                    out=qt_sb[:, SQ * p : SQ * (p + 1)],
                    in_=qt.ap()[:, SQ * p : SQ * (p + 1)],
                )
                nc.sync.dma_start(
                    out=kt_sb[:, S * p : S * (p + 1)],
                    in_=kt.ap()[:, S * p : S * (p + 1)],
                )
                nc.sync.dma_start(
                    out=vt_sb[:, S * p : S * (p + 1)],
                    in_=vt.ap()[:, S * p : S * (p + 1)],
                )

            nc.gpsimd.memset(vp_all[:], 1.0)
            nc.gpsimd.memset(qpT0[:], 0.0)
            nc.gpsimd.memset(qpT1[:], 0.0)

            for p in range(NPAIR):
                ps = ps1.tile([128, 512], F32, tag="ps1", name=f"psqp{p}")
                nc.tensor.matmul(
                    ps[:],
                    lhsT=wq_sb[:, 128 * p : 128 * (p + 1)],
                    rhs=qt_sb[:, SQ * p : SQ * (p + 1)],
                    start=True,
                    stop=True,
                )
                nc.scalar.activation(
                    qpT0[0:64, SQ * p : SQ * (p + 1)], ps[0:64, :], AF.Identity,
                    bias=bq_sb[0:64, p : p + 1],
                )
                nc.scalar.activation(
                    qpT1[64:128, SQ * p : SQ * (p + 1)], ps[64:128, :], AF.Identity,
                    bias=bq_sb[64:128, p : p + 1],
                )
                for n in range(4):
                    ps = ps1.tile([128, 512], F32, tag="ps1", name=f"pskp{p}_{n}")
                    nc.tensor.matmul(
                        ps[:],
                        lhsT=wk_sb[:, 128 * p : 128 * (p + 1)],
                        rhs=kt_sb[:, S * p + 512 * n : S * p + 512 * (n + 1)],
                        start=True,
                        stop=True,
                    )
                    nc.scalar.activation(
                        kpT[:, S * p + 512 * n : S * p + 512 * (n + 1)], ps[:],
                        AF.Identity, bias=bk_sb[:, p : p + 1],
                    )
                for g in range(4):
                    ps = ps1.tile([128, 512], F32, tag="ps1", name=f"psvp{p}_{g}")
                    for jj in range(4):
                        j = 4 * g + jj
                        nc.tensor.matmul(
                            ps[:, 128 * jj : 128 * (jj + 1)],
                            lhsT=vt_sb[:, S * p + 128 * j : S * p + 128 * (j + 1)],
                            rhs=wv_sb[:, 128 * p : 128 * (p + 1)],
                            start=True,
                            stop=True,
                        )
                    src = ps[:].rearrange("x (jj u d) -> x u jj d", jj=4, u=2)
                    dst = vp_all[:].rearrange("x (h j c) -> x h j c", h=H, j=16)[
                        :, 2 * p : 2 * p + 2, 4 * g : 4 * g + 4, 0:64
                    ]
                    nc.vector.tensor_copy(dst, src)

        # ---------------- phase 2: attention ----------------
        # Exact-causal column skipping at 128-col granularity: for ktile j,
        # packed q-columns below AV_OFF[j] are provably masked for every
        # core, so neither the score matmul, the exp, nor the AV matmul
        # touches them. The remaining partial-diagonal region is zeroed by
        # the host-supplied multiplicative mask.
        AV_OFF = [0] * 4 + [128] * 4 + [256] * 4 + [384] * 4
        with ExitStack() as ex2:
            p2 = ex2.enter_context(tc.tile_pool(name="p2", bufs=1))
            denom = p2.tile([33, NPAIR * SQ], F32)
            nc.gpsimd.memset(denom[:], 1.0)
            mask_sb = p2.tile([128, 16 * 64], BF16)
            nc.sync.dma_start(out=mask_sb[:], in_=maskin.ap())
            epool = ex2.enter_context(tc.tile_pool(name="epool", bufs=2))
            psS = ex2.enter_context(tc.tile_pool(name="psS", bufs=3, space="PSUM"))
            psO = ex2.enter_context(tc.tile_pool(name="psO", bufs=2, space="PSUM"))

            mview = mask_sb[:].rearrange("x (j q) -> x j q", j=16)  # (128,16,64)

            def scores_block(h):
                # eT storage is left-aligned per ktile: column 512*j + x holds
                # the exp'd score for packed q-col 32*j0 + x (j0 = j & ~1), so
                # every downstream access pattern is a regular 512-stride view.
                p, u = divmod(h, 2)
                eT = epool.tile([128, 16 * 512], BF16, tag="eT", name=f"eT{h}")
                ev = eT[:].rearrange("x (j q) -> x j q", j=16)
                qv = qpT0 if u == 0 else qpT1
                for g in range(8):
                    j0 = 2 * g
                    N = 512 - 32 * j0
                    pss = psS.tile([128, 1024], F32, tag="psS", name=f"pss{h}_{g}")
                    for jj in range(2):
                        j = j0 + jj
                        nc.tensor.matmul(
                            pss[:, N * jj : N * (jj + 1)],
                            lhsT=kpT[:, S * p + 128 * j : S * p + 128 * (j + 1)],
                            rhs=qv[:, SQ * p + 32 * j0 : SQ * p + 512],
                            start=True,
                            stop=True,
                        )
                    nc.scalar.activation(
                        ev[:, j0 : j0 + 2, 0:N],
                        pss[:, 0 : 2 * N].rearrange("x (t q) -> x t q", t=2),
                        AF.Exp,
                        scale=1.0 / DK,
                    )
                # one fused mask op: pad+diagonal strip = first 64 cols per ktile
                nc.vector.tensor_mul(
                    ev[:, :, 0:64], ev[:, :, 0:64], mview[:, :, :]
                )
                return eT

            def av_block(h, eT):
                p, u = divmod(h, 2)
                pso = psO.tile([65, 512], F32, tag="psO", name=f"pso{h}")
                for j in range(16):
                    off = 32 * (j & ~1)
                    nc.tensor.matmul(
                        pso[:, off:512],
                        lhsT=vp_all[:, h * 1040 + 65 * j : h * 1040 + 65 * (j + 1)],
                        rhs=eT[:, 512 * j : 512 * j + 512 - off],
                        start=(j == 0),
                        stop=(j == 15),
                        skip_group_check=True,
                    )
                nc.vector.tensor_copy(
                    OT[64 * u : 64 * (u + 1), SQ * p : SQ * (p + 1)], pso[0:64, :]
                )
                nc.vector.tensor_copy(
                    denom[32 * u : 32 * u + 1, SQ * p : SQ * (p + 1)], pso[64:65, :]
                )

            denom_bf = p2.tile([33, NPAIR * SQ], BF16)

            def divide_pair(p):
                # per-pair softmax division, unblocks fc contraction chunk p
                dsl = denom[0:33, SQ * p : SQ * (p + 1)]
                nc.vector.reciprocal_approx_fast(dsl, dsl)
                dbf = denom_bf[0:33, SQ * p : SQ * (p + 1)]
                nc.vector.tensor_copy(dbf, dsl)
                psb = psO.tile([128, 512], F32, tag="psO", name=f"psb{p}")
                nc.tensor.matmul(
                    psb[:], lhsT=blk2_sb[:], rhs=dbf, start=True, stop=True
                )
                sl = OT[:, SQ * p : SQ * (p + 1)]
                nc.vector.tensor_mul(sl, sl, psb[:])
                nc.vector.tensor_scalar_add(sl, sl, bv_sb[:, p : p + 1])

            pipe = []
            for h in range(H):
                pipe.append((h, scores_block(h)))
                if len(pipe) > 1:
                    hh, eTT = pipe.pop(0)
                    av_block(hh, eTT)
                    if hh % 2 == 1:
                        divide_pair(hh // 2)
            for hh, eTT in pipe:
                av_block(hh, eTT)
                if hh % 2 == 1:
                    divide_pair(hh // 2)

        exA.close()

        # ---------------- phase 3: fc + residual + LN ----------------
        with ExitStack() as ex3:
            p3 = ex3.enter_context(tc.tile_pool(name="p3", bufs=1))
            xt = p3.tile([128, 4 * E], F32)
            Ab = p3.tile([128, E], F32)
            Bb = p3.tile([128, E], F32)
            gamma_sb = p3.tile([128, 4], F32)
            beta_sb = p3.tile([128, 4], F32)
            stat_sb = p3.tile([1, 2 * E], F32)
            stat2_sb = p3.tile([1, 2 * E], F32)
            ln128 = p3.tile([128, E], F32)
            vrp = ex3.enter_context(tc.tile_pool(name="vrp", bufs=2))
            psF = ex3.enter_context(tc.tile_pool(name="psF", bufs=4, space="PSUM"))
            psT = ex3.enter_context(tc.tile_pool(name="psT", bufs=4, space="PSUM"))
            dramp = ex3.enter_context(tc.tile_pool(name="dramp", bufs=1, space="DRAM"))
            ar_in = dramp.tile([1, 2 * E], F32)
            ar_out = dramp.tile([1, 2 * E], F32)

            nc.sync.dma_start(out=gamma_sb[:], in_=gamma.ap())
            nc.sync.dma_start(out=beta_sb[:], in_=beta.ap())

            pstats = [psT.tile([1, 512], F32, tag="psT", name=f"pst{t}") for t in range(4)]
            for i in range(4):
                for nh in range(2):
                    psf = psF.tile([128, 512], F32, tag="psF", name=f"psf{i}_{nh}")
                    for kc in range(8):
                        nc.tensor.matmul(
                            psf[:],
                            lhsT=OT[:, SQ * kc + 128 * i : SQ * kc + 128 * (i + 1)],
                            rhs=wfc_sb[:, E * kc + 512 * nh : E * kc + 512 * (nh + 1)],
                            start=(kc == 0),
                            stop=(kc == 7),
                        )
                    nc.scalar.copy(
                        xt[:, E * i + 512 * nh : E * i + 512 * (nh + 1)], psf[:]
                    )
                vr = vrp.tile([128, E], F32, tag="vr", name=f"vr{i}")
                nc.sync.dma_start(out=vr[:], in_=vres.ap()[i])
                xi = xt[:, E * i : E * (i + 1)]
                nc.vector.tensor_add(xi, xi, vr[:])
                xsq = vrp.tile([128, E], F32, tag="xsq", name=f"xsq{i}")
                nc.vector.tensor_mul(xsq[:], xi, xi)
                for nh in range(2):
                    nc.tensor.matmul(
                        pstats[nh][:],
                        lhsT=ones_col_sb[:],
                        rhs=xt[:, E * i + 512 * nh : E * i + 512 * (nh + 1)],
                        start=(i == 0),
                        stop=(i == 3),
                    )
                    nc.tensor.matmul(
                        pstats[2 + nh][:],
                        lhsT=ones_col_sb[:],
                        rhs=xsq[:, 512 * nh : 512 * (nh + 1)],
                        start=(i == 0),
                        stop=(i == 3),
                    )
            for nh in range(2):
                nc.vector.tensor_copy(
                    stat_sb[0:1, 512 * nh : 512 * (nh + 1)], pstats[nh][:]
                )
                nc.vector.tensor_copy(
                    stat_sb[0:1, E + 512 * nh : E + 512 * (nh + 1)], pstats[2 + nh][:]
                )
            nc.sync.dma_start(out=ar_in[:], in_=stat_sb[:])
            nc.gpsimd.collective_compute(
                "AllReduce",
                mybir.AluOpType.add,
                replica_groups=GROUPS,
                ins=[ar_in.opt()],
                outs=[ar_out.opt()],
            )
            nc.sync.dma_start(out=stat2_sb[:], in_=ar_out[:])
            # broadcast raw sums to all partitions FIRST, then do the whole
            # LN scalar chain with 128-lane parallelism (Ab=mean, Bb=meansq)
            for row, dst in ((0, Ab), (1, Bb)):
                for nh in range(2):
                    ps = psF.tile([128, 512], F32, tag="psF", name=f"psbc{row}_{nh}")
                    nc.tensor.matmul(
                        ps[:],
                        lhsT=ones_row_sb[:],
                        rhs=stat2_sb[0:1, E * row + 512 * nh : E * row + 512 * (nh + 1)],
                        start=True,
                        stop=True,
                    )
                    nc.scalar.mul(dst[:, 512 * nh : 512 * (nh + 1)], ps[:], 1.0 / S)
            # var = meansq - mean^2 ; A = rsqrt(var+eps) ; B = -mean*A
            nc.vector.tensor_mul(ln128[:], Ab[:], Ab[:])
            nc.vector.tensor_sub(Bb[:], Bb[:], ln128[:])
            nc.scalar.activation(Bb[:], Bb[:], AF.Sqrt, bias=eps_sb_b[:])
            nc.vector.reciprocal_approx_fast(Bb[:], Bb[:])
            nc.vector.tensor_mul(ln128[:], Ab[:], Bb[:])
            nc.scalar.mul(ln128[:], ln128[:], -1.0)
            # now: A (rstd) lives in Bb, B lives in ln128 -> swap names below
            for i in range(4):
                sl = xt[:, E * i : E * (i + 1)]
                nc.vector.tensor_mul(sl, sl, Bb[:])
                nc.vector.tensor_add(sl, sl, ln128[:])
                nc.scalar.activation(
                    sl, sl, AF.Identity,
                    bias=beta_sb[:, i : i + 1], scale=gamma_sb[:, i : i + 1],
                )
                nc.sync.dma_start(out=out.ap()[i], in_=sl)


def build():
    nc = bacc.Bacc("TRN2", target_bir_lowering=False, debug=False, num_devices=8)
    _emit(nc)
    nc.compile()
    return nc


def _masks():
    global _MASKS
    if _MASKS is None:
        kk = np.arange(128)[:, None]
        x = np.arange(64)[None, :]
        ms = []
        for r in range(4):
            m = np.zeros((128, 16 * 64), np.float32)
            for j in range(16):
                c = 32 * (j & ~1) + x  # packed q-column
                q = np.where(c < 256, 4 * c + r, 1024 + 4 * (c - 256) + r)
                m[:, 64 * j : 64 * (j + 1)] = kk <= (q - 128 * j)
            ms.append(m.astype(NPBF16))
        _MASKS = ms
    return _MASKS


def _blockdiag(w):
    # (16, 64, 64) f32 -> (8, 128, 128) bf16 per-pair block diagonal
    o = np.zeros((NPAIR, 128, 128), np.float32)
    for p in range(NPAIR):
        o[p, :64, :64] = w[2 * p]
        o[p, 64:, 64:] = w[2 * p + 1]
    return o.astype(NPBF16)


def kernel(**inputs):
    global _NC_CACHE
    q = np.asarray(inputs["q"], np.float32)
    k = np.asarray(inputs["k"], np.float32)
    v = np.asarray(inputs["v"], np.float32)
    Wq = np.asarray(inputs["Wq"], np.float32)
    Wk = np.asarray(inputs["Wk"], np.float32)
    Wv = np.asarray(inputs["Wv"], np.float32)
    bq = np.asarray(inputs["bq"], np.float32)
    bk = np.asarray(inputs["bk"], np.float32)
    bv = np.asarray(inputs["bv"], np.float32)
    Wfc = np.asarray(inputs["Wfc"], np.float32)
    bfc = np.asarray(inputs["bfc"], np.float32)  # noqa: F841  cancels in LN
    gamma = np.asarray(inputs["gamma"], np.float32)
    beta = np.asarray(inputs["beta"], np.float32)

    if _NC_CACHE is None:
        _NC_CACHE = build()
    nc = _NC_CACHE
    masks = _masks()

    wq_h = np.ascontiguousarray(_blockdiag(Wq).transpose(1, 0, 2).reshape(128, -1))
    wk_h = np.ascontiguousarray(_blockdiag(Wk).transpose(1, 0, 2).reshape(128, -1))
    wv_h = np.ascontiguousarray(_blockdiag(Wv).transpose(1, 0, 2).reshape(128, -1))
    bq_h = np.ascontiguousarray(bq.reshape(NPAIR, 128).T)
    bk_h = np.ascontiguousarray(bk.reshape(NPAIR, 128).T)
    bv_h = np.ascontiguousarray(bv.reshape(NPAIR, 128).T)
    wfc_h = np.ascontiguousarray(Wfc.reshape(8, 128, E).transpose(1, 0, 2).reshape(128, -1))

    def _tile8(a):  # (S, E) -> transposed, pair-tiled (128, 8*S)
        t = a.T.reshape(NPAIR, 128, -1).transpose(1, 0, 2)
        return np.ascontiguousarray(t.reshape(128, -1))

    kts = [_tile8(k[b]).astype(NPBF16) for b in range(B)]
    vts = [_tile8(v[b]).astype(NPBF16) for b in range(B)]
    qts = [q[b].T for b in range(B)]

    in_maps = []
    for c in range(8):
        b, r = divmod(c, 4)
        in_maps.append(
            {
                "qt": np.ascontiguousarray(
                    qts[b][:, r::4].reshape(NPAIR, 128, SQ).transpose(1, 0, 2)
                    .reshape(128, -1)
                ).astype(NPBF16),
                "kt": kts[b],
                "vt": vts[b],
                "wq": wq_h,
                "wk": wk_h,
                "wv": wv_h,
                "bq": bq_h,
                "bk": bk_h,
                "bv": bv_h,
                "wfc": wfc_h,
                "vres": np.ascontiguousarray(v[b, r::4, :]).reshape(4, 128, E),
                "gamma": np.ascontiguousarray(gamma[r::4].reshape(4, 128).T),
                "beta": np.ascontiguousarray(beta[r::4].reshape(4, 128).T),
                "mask": masks[r],
            }
        )

    global _last_in_maps
    _last_in_maps = in_maps
    res = run_bass_kernel_spmd(nc, in_maps, list(range(8))).results
    full = np.empty((B, S, E), np.float32)
    for c in range(8):
        b, r = divmod(c, 4)
        full[b, r::4, :] = res[c]["out"].reshape(SQ, E)
    return full


# revision 25
# speedup vs baseline: 1.0977x; 1.0977x over previous
"""Trainium2 Bass kernel for nn_MultiHeadAttention (sparse_attention).

Sharding: 8 cores = 2 batches x 4-way sequence split. Core c handles
batch b=c//4 and q-columns r::4 (r=c%4) of that batch -- a perfectly
balanced, SPMD-uniform causal split. Each core computes all 16 heads
for its 512 q positions (QKV projections for full S are replicated
within a batch group), the fc projection fully locally (K=1024), and
only an 8KB AllReduce of LayerNorm statistics crosses cores.

Layout: everything feature-on-partition / sequence-on-free.
  qpT/kpT: (dk, q) bf16;  vp: (s, dk) bf16 with a ones-column appended
  (the ones row of the AV output is the softmax denominator).
Scores are computed transposed (k on partitions, q on free) so softmax
denominators come free from the AV matmul and no transposes are needed
anywhere. Causality is enforced by data-driven multiplicative masks
(host-computed per core) so the instruction stream is identical on all
8 cores. Softmax needs no max-subtraction: scores are (qp.kp)/64 with
|s| < ~0.05, so exp() cannot overflow.
"""

import sys

for _p in ("/opt/trn_rl_repo",):
    if _p not in sys.path:
        sys.path.insert(0, _p)

from contextlib import ExitStack

import ml_dtypes
import numpy as np

import concourse.bacc as bacc
import concourse.tile as tile
from concourse import mybir
from concourse.bass_utils import run_bass_kernel_spmd

BF16 = mybir.dt.bfloat16
F32 = mybir.dt.float32
F32R = mybir.dt.float32r
NPBF16 = ml_dtypes.bfloat16
AF = mybir.ActivationFunctionType

B, S, E, H, DK = 2, 2048, 1024, 16, 64
NPAIR = 8  # head pairs
SQ = 512  # q columns per core
EPS = 1e-4
GROUPS = [[0, 1, 2, 3], [4, 5, 6, 7]]

_NC_CACHE = None
_MASKS = None


def _emit(nc):
    qt = nc.dram_tensor("qt", [128, NPAIR * SQ], BF16, kind="ExternalInput")
    kt = nc.dram_tensor("kt", [128, NPAIR * S], BF16, kind="ExternalInput")
    vt = nc.dram_tensor("vt", [128, NPAIR * S], BF16, kind="ExternalInput")
    wq = nc.dram_tensor("wq", [128, NPAIR * 128], BF16, kind="ExternalInput")
    wk = nc.dram_tensor("wk", [128, NPAIR * 128], BF16, kind="ExternalInput")
    wv = nc.dram_tensor("wv", [128, NPAIR * 128], BF16, kind="ExternalInput")
    bq = nc.dram_tensor("bq", [128, NPAIR], F32, kind="ExternalInput")
    bk = nc.dram_tensor("bk", [128, NPAIR], F32, kind="ExternalInput")
    bv = nc.dram_tensor("bv", [128, NPAIR], F32, kind="ExternalInput")
    wfc = nc.dram_tensor("wfc", [128, 8 * E], F32R, kind="ExternalInput")
    vres = nc.dram_tensor("vres", [4, 128, E], F32, kind="ExternalInput")
    gamma = nc.dram_tensor("gamma", [128, 4], F32, kind="ExternalInput")
    beta = nc.dram_tensor("beta", [128, 4], F32, kind="ExternalInput")
    maskin = nc.dram_tensor("mask", [128, 16 * 64], BF16, kind="ExternalInput")
    out = nc.dram_tensor("out", [4, 128, E], F32, kind="ExternalOutput")

    ones_col_c = nc.inline_tensor(np.ones((128, 1), np.float32), "ones_col")
    ones_row_c = nc.inline_tensor(np.ones((1, 128), np.float32), "ones_row")
    blk2_np = np.zeros((33, 128), np.float32)
    blk2_np[0, :64] = 1.0
    blk2_np[32, 64:] = 1.0
    blk2_c = nc.inline_tensor(blk2_np.astype(ml_dtypes.bfloat16), "blk2")
    eps_c = nc.inline_tensor(np.full((1, 1), EPS, np.float32), "epsc")
    eps_b_c = nc.inline_tensor(np.full((128, 1), EPS, np.float32), "epsbc")

    with tile.TileContext(nc) as tc, ExitStack() as ex:
        cst = ex.enter_context(tc.tile_pool(name="cst", bufs=1))
        ones_col_sb = cst.tile([128, 1], F32)
        ones_row_sb = cst.tile([1, 128], F32)
        blk2_sb = cst.tile([33, 128], BF16)
        bq_sb = cst.tile([128, NPAIR], F32)
        bk_sb = cst.tile([128, NPAIR], F32)
        bv_sb = cst.tile([128, NPAIR], F32)
        eps_sb = cst.tile([1, 1], F32)
        eps_sb_b = cst.tile([128, 1], F32)
        nc.sync.dma_start(out=ones_col_sb[:], in_=ones_col_c.ap())
        nc.sync.dma_start(out=ones_row_sb[:], in_=ones_row_c.ap())
        nc.sync.dma_start(out=blk2_sb[:], in_=blk2_c.ap())
        nc.sync.dma_start(out=eps_sb[:], in_=eps_c.ap())
        nc.sync.dma_start(out=eps_sb_b[:], in_=eps_b_c.ap())
        nc.sync.dma_start(out=bq_sb[:], in_=bq.ap())
        nc.sync.dma_start(out=bk_sb[:], in_=bk.ap())
        nc.sync.dma_start(out=bv_sb[:], in_=bv.ap())

        dramw = ex.enter_context(tc.tile_pool(name="dramw", bufs=1, space="DRAM"))
        warm_in = dramw.tile([1, 16], F32)
        warm_out = dramw.tile([1, 16], F32)
        warm_sb = ex.enter_context(tc.tile_pool(name="warmp", bufs=1)).tile([1, 16], F32)
        nc.vector.memset(warm_sb[:], 0.0)
        nc.sync.dma_start(out=warm_in[:], in_=warm_sb[:])
        nc.gpsimd.collective_compute(
            "AllReduce",
            mybir.AluOpType.add,
            replica_groups=GROUPS,
            ins=[warm_in.opt()],
            outs=[warm_out.opt()],
        )
        # live through phase 3
        poolC = ex.enter_context(tc.tile_pool(name="poolC", bufs=1))
        OT = poolC.tile([128, NPAIR * SQ], F32R)
        wfc_sb = poolC.tile([128, 8 * E], F32R)
        nc.sync.dma_start(out=wfc_sb[:], in_=wfc.ap())
        # live through phase 2
        exA = ex.enter_context(ExitStack())
        poolA = exA.enter_context(tc.tile_pool(name="poolA", bufs=1))
        qpT0 = poolA.tile([128, NPAIR * SQ], BF16)
        qpT1 = poolA.tile([128, NPAIR * SQ], BF16)
        kpT = poolA.tile([128, NPAIR * S], BF16)
        vp_all = poolA.tile([128, H * 16 * 65], BF16)

        exPS = ex.enter_context(ExitStack())
        psS = exPS.enter_context(tc.tile_pool(name="psS", bufs=3, space="PSUM"))
        psO = exPS.enter_context(tc.tile_pool(name="psO", bufs=2, space="PSUM"))

        # ---------------- phase 1: load + projections ----------------
        with ExitStack() as ex1:
            p1 = ex1.enter_context(tc.tile_pool(name="p1", bufs=1))
            qt_sb = p1.tile([128, NPAIR * SQ], BF16)
            kt_sb = p1.tile([128, NPAIR * S], BF16)
            vt_sb = p1.tile([128, NPAIR * S], BF16)
            wq_sb = p1.tile([128, NPAIR * 128], BF16)
            wk_sb = p1.tile([128, NPAIR * 128], BF16)
            wv_sb = p1.tile([128, NPAIR * 128], BF16)

            for w_in, w_sb in ((wq, wq_sb), (wk, wk_sb), (wv, wv_sb)):
                nc.sync.dma_start(out=w_sb[:], in_=w_in.ap())
            for p in range(NPAIR):
                nc.sync.dma_start(
                    out=qt_sb[:, SQ * p : SQ * (p + 1)],
                    in_=qt.ap()[:, SQ * p : SQ * (p + 1)],
                )
                nc.sync.dma_start(
                    out=kt_sb[:, S * p : S * (p + 1)],
                    in_=kt.ap()[:, S * p : S * (p + 1)],
                )
                nc.sync.dma_start(
                    out=vt_sb[:, S * p : S * (p + 1)],
                    in_=vt.ap()[:, S * p : S * (p + 1)],
                )

            nc.gpsimd.memset(vp_all[:], 1.0)
            nc.gpsimd.memset(qpT0[:], 0.0)
            nc.gpsimd.memset(qpT1[:], 0.0)

            for p in range(NPAIR):
                ps = psS.tile([128, 1024], F32, tag="psS", name=f"psqp{p}")[:, 0:512]
                nc.tensor.matmul(
                    ps[:],
                    lhsT=wq_sb[:, 128 * p : 128 * (p + 1)],
                    rhs=qt_sb[:, SQ * p : SQ * (p + 1)],
                    start=True,
                    stop=True,
                )
                nc.scalar.activation(
                    qpT0[0:64, SQ * p : SQ * (p + 1)], ps[0:64, :], AF.Identity,
                    bias=bq_sb[0:64, p : p + 1],
                )
                nc.scalar.activation(
                    qpT1[64:128, SQ * p : SQ * (p + 1)], ps[64:128, :], AF.Identity,
                    bias=bq_sb[64:128, p : p + 1],
                )
                for n in range(4):
                    ps = psS.tile([128, 1024], F32, tag="psS", name=f"pskp{p}_{n}")[:, 0:512]
                    nc.tensor.matmul(
                        ps[:],
                        lhsT=wk_sb[:, 128 * p : 128 * (p + 1)],
                        rhs=kt_sb[:, S * p + 512 * n : S * p + 512 * (n + 1)],
                        start=True,
                        stop=True,
                    )
                    nc.scalar.activation(
                        kpT[:, S * p + 512 * n : S * p + 512 * (n + 1)], ps[:],
                        AF.Identity, bias=bk_sb[:, p : p + 1],
                    )
                for g in range(4):
                    ps = psS.tile([128, 1024], F32, tag="psS", name=f"psvp{p}_{g}")[:, 0:512]
                    for jj in range(4):
                        j = 4 * g + jj
                        nc.tensor.matmul(
                            ps[:, 128 * jj : 128 * (jj + 1)],
                            lhsT=vt_sb[:, S * p + 128 * j : S * p + 128 * (j + 1)],
                            rhs=wv_sb[:, 128 * p : 128 * (p + 1)],
                            start=True,
                            stop=True,
                        )
                    src = ps[:].rearrange("x (jj u d) -> x u jj d", jj=4, u=2)
                    dst = vp_all[:].rearrange("x (h j c) -> x h j c", h=H, j=16)[
                        :, 2 * p : 2 * p + 2, 4 * g : 4 * g + 4, 0:64
                    ]
                    nc.vector.tensor_copy(dst, src)

        # ---------------- phase 2: attention ----------------
        # Exact-causal column skipping at 128-col granularity: for ktile j,
        # packed q-columns below AV_OFF[j] are provably masked for every
        # core, so neither the score matmul, the exp, nor the AV matmul
        # touches them. The remaining partial-diagonal region is zeroed by
        # the host-supplied multiplicative mask.
        AV_OFF = [0] * 4 + [128] * 4 + [256] * 4 + [384] * 4
        with ExitStack() as ex2:
            p2 = ex2.enter_context(tc.tile_pool(name="p2", bufs=1))
            denom = p2.tile([33, NPAIR * SQ], F32)
            nc.gpsimd.memset(denom[:], 1.0)
            mask_sb = p2.tile([128, 16 * 64], BF16)
            nc.sync.dma_start(out=mask_sb[:], in_=maskin.ap())
            epool = ex2.enter_context(tc.tile_pool(name="epool", bufs=2))

            mview = mask_sb[:].rearrange("x (j q) -> x j q", j=16)  # (128,16,64)

            def scores_block(h):
                # eT storage is left-aligned per ktile: column 512*j + x holds
                # the exp'd score for packed q-col 32*j0 + x (j0 = j & ~1), so
                # every downstream access pattern is a regular 512-stride view.
                p, u = divmod(h, 2)
                eT = epool.tile([128, 16 * 512], BF16, tag="eT", name=f"eT{h}")
                ev = eT[:].rearrange("x (j q) -> x j q", j=16)
                qv = qpT0 if u == 0 else qpT1
                for g in range(8):
                    j0 = 2 * g
                    N = 512 - 32 * j0
                    pss = psS.tile([128, 1024], F32, tag="psS", name=f"pss{h}_{g}")
                    for jj in range(2):
                        j = j0 + jj
                        nc.tensor.matmul(
                            pss[:, N * jj : N * (jj + 1)],
                            lhsT=kpT[:, S * p + 128 * j : S * p + 128 * (j + 1)],
                            rhs=qv[:, SQ * p + 32 * j0 : SQ * p + 512],
                            start=True,
                            stop=True,
                        )
                    nc.scalar.activation(
                        ev[:, j0 : j0 + 2, 0:N],
                        pss[:, 0 : 2 * N].rearrange("x (t q) -> x t q", t=2),
                        AF.Exp,
                        scale=1.0 / DK,
                    )
                # one fused mask op: pad+diagonal strip = first 64 cols per ktile
                nc.vector.tensor_mul(
                    ev[:, :, 0:64], ev[:, :, 0:64], mview[:, :, :]
                )
                return eT

            def av_block(h, eT):
                p, u = divmod(h, 2)
                pso = psO.tile([65, 512], F32, tag="psO", name=f"pso{h}")
                for j in range(16):
                    off = 32 * (j & ~1)
                    nc.tensor.matmul(
                        pso[:, off:512],
                        lhsT=vp_all[:, h * 1040 + 65 * j : h * 1040 + 65 * (j + 1)],
                        rhs=eT[:, 512 * j : 512 * j + 512 - off],
                        start=(j == 0),
                        stop=(j == 15),
                        skip_group_check=True,
                    )
                nc.vector.tensor_copy(
                    OT[64 * u : 64 * (u + 1), SQ * p : SQ * (p + 1)], pso[0:64, :]
                )
                nc.vector.tensor_copy(
                    denom[32 * u : 32 * u + 1, SQ * p : SQ * (p + 1)], pso[64:65, :]
                )

            denom_bf = p2.tile([33, NPAIR * SQ], BF16)

            def divide_pair(p):
                # per-pair softmax division, unblocks fc contraction chunk p
                dsl = denom[0:33, SQ * p : SQ * (p + 1)]
                nc.vector.reciprocal_approx_fast(dsl, dsl)
                dbf = denom_bf[0:33, SQ * p : SQ * (p + 1)]
                nc.vector.tensor_copy(dbf, dsl)
                psb = psO.tile([128, 512], F32, tag="psO", name=f"psb{p}")
                nc.tensor.matmul(
                    psb[:], lhsT=blk2_sb[:], rhs=dbf, start=True, stop=True
                )
                sl = OT[:, SQ * p : SQ * (p + 1)]
                nc.vector.tensor_mul(sl, sl, psb[:])
                nc.vector.tensor_scalar_add(sl, sl, bv_sb[:, p : p + 1])

            pipe = []
            for h in range(H):
                pipe.append((h, scores_block(h)))
                if len(pipe) > 1:
                    hh, eTT = pipe.pop(0)
                    av_block(hh, eTT)
                    if hh % 2 == 1:
                        divide_pair(hh // 2)
            for hh, eTT in pipe:
                av_block(hh, eTT)
                if hh % 2 == 1:
                    divide_pair(hh // 2)

        exA.close()
        exPS.close()

        # ---------------- phase 3: fc + residual + LN ----------------
        with ExitStack() as ex3:
            p3 = ex3.enter_context(tc.tile_pool(name="p3", bufs=1))
            xt = p3.tile([128, 4 * E], F32)
            Ab = p3.tile([128, E], F32)
            Bb = p3.tile([128, E], F32)
            gamma_sb = p3.tile([128, 4], F32)
            beta_sb = p3.tile([128, 4], F32)
            stat_sb = p3.tile([1, 2 * E], F32)
            stat2_sb = p3.tile([1, 2 * E], F32)
            ln128 = p3.tile([128, E], F32)
            vrp = ex3.enter_context(tc.tile_pool(name="vrp", bufs=2))
            psF = ex3.enter_context(tc.tile_pool(name="psF", bufs=4, space="PSUM"))
            psT = ex3.enter_context(tc.tile_pool(name="psT", bufs=4, space="PSUM"))
            dramp = ex3.enter_context(tc.tile_pool(name="dramp", bufs=1, space="DRAM"))
            ar_in = dramp.tile([1, 2 * E], F32)
            ar_out = dramp.tile([1, 2 * E], F32)

            nc.sync.dma_start(out=gamma_sb[:], in_=gamma.ap())
            nc.sync.dma_start(out=beta_sb[:], in_=beta.ap())

            pstats = [psT.tile([1, 512], F32, tag="psT", name=f"pst{t}") for t in range(4)]
            for i in range(4):
                for nh in range(2):
                    psf = psF.tile([128, 512], F32, tag="psF", name=f"psf{i}_{nh}")
                    for kc in range(8):
                        nc.tensor.matmul(
                            psf[:],
                            lhsT=OT[:, SQ * kc + 128 * i : SQ * kc + 128 * (i + 1)],
                            rhs=wfc_sb[:, E * kc + 512 * nh : E * kc + 512 * (nh + 1)],
                            start=(kc == 0),
                            stop=(kc == 7),
                        )
                    nc.scalar.copy(
                        xt[:, E * i + 512 * nh : E * i + 512 * (nh + 1)], psf[:]
                    )
                vr = vrp.tile([128, E], F32, tag="vr", name=f"vr{i}")
                nc.sync.dma_start(out=vr[:], in_=vres.ap()[i])
                xi = xt[:, E * i : E * (i + 1)]
                nc.vector.tensor_add(xi, xi, vr[:])
                xsq = vrp.tile([128, E], F32, tag="xsq", name=f"xsq{i}")
                nc.vector.tensor_mul(xsq[:], xi, xi)
                for nh in range(2):
                    nc.tensor.matmul(
                        pstats[nh][:],
                        lhsT=ones_col_sb[:],
                        rhs=xt[:, E * i + 512 * nh : E * i + 512 * (nh + 1)],
                        start=(i == 0),
                        stop=(i == 3),
                    )
                    nc.tensor.matmul(
                        pstats[2 + nh][:],
                        lhsT=ones_col_sb[:],
                        rhs=xsq[:, 512 * nh : 512 * (nh + 1)],
                        start=(i == 0),
                        stop=(i == 3),
                    )
            for nh in range(2):
                nc.vector.tensor_copy(
                    stat_sb[0:1, 512 * nh : 512 * (nh + 1)], pstats[nh][:]
                )
                nc.vector.tensor_copy(
                    stat_sb[0:1, E + 512 * nh : E + 512 * (nh + 1)], pstats[2 + nh][:]
                )
            nc.sync.dma_start(out=ar_in[:], in_=stat_sb[:])
            nc.gpsimd.collective_compute(
                "AllReduce",
                mybir.AluOpType.add,
                replica_groups=GROUPS,
                ins=[ar_in.opt()],
                outs=[ar_out.opt()],
            )
            nc.sync.dma_start(out=stat2_sb[:], in_=ar_out[:])
            # broadcast raw sums to all partitions FIRST, then do the whole
            # LN scalar chain with 128-lane parallelism (Ab=mean, Bb=meansq)
            for row, dst in ((0, Ab), (1, Bb)):
                for nh in range(2):
                    ps = psF.tile([128, 512], F32, tag="psF", name=f"psbc{row}_{nh}")
                    nc.tensor.matmul(
                        ps[:],
                        lhsT=ones_row_sb[:],
                        rhs=stat2_sb[0:1, E * row + 512 * nh : E * row + 512 * (nh + 1)],
                        start=True,
                        stop=True,
                    )
                    nc.scalar.mul(dst[:, 512 * nh : 512 * (nh + 1)], ps[:], 1.0 / S)
            # var = meansq - mean^2 ; A = rsqrt(var+eps) ; B = -mean*A
            nc.vector.tensor_mul(ln128[:], Ab[:], Ab[:])
            nc.vector.tensor_sub(Bb[:], Bb[:], ln128[:])
            nc.scalar.activation(Bb[:], Bb[:], AF.Sqrt, bias=eps_sb_b[:])
            nc.vector.reciprocal_approx_fast(Bb[:], Bb[:])
            nc.vector.tensor_mul(ln128[:], Ab[:], Bb[:])
            nc.scalar.mul(ln128[:], ln128[:], -1.0)
            # now: A (rstd) lives in Bb, B lives in ln128 -> swap names below
            for i in range(4):
                sl = xt[:, E * i : E * (i + 1)]
                nc.vector.tensor_mul(sl, sl, Bb[:])
                nc.vector.tensor_add(sl, sl, ln128[:])
                nc.scalar.activation(
                    sl, sl, AF.Identity,
                    bias=beta_sb[:, i : i + 1], scale=gamma_sb[:, i : i + 1],
                )
                nc.sync.dma_start(out=out.ap()[i], in_=sl)


def build():
    nc = bacc.Bacc("TRN2", target_bir_lowering=False, debug=False, num_devices=8)
    _emit(nc)
    nc.compile()
    return nc


def _masks():
    global _MASKS
    if _MASKS is None:
        kk = np.arange(128)[:, None]
        x = np.arange(64)[None, :]
        ms = []
        for r in range(4):
            m = np.zeros((128, 16 * 64), np.float32)
            for j in range(16):
                c = 32 * (j & ~1) + x  # packed q-column
                q = np.where(c < 256, 4 * c + r, 1024 + 4 * (c - 256) + r)
                m[:, 64 * j : 64 * (j + 1)] = kk <= (q - 128 * j)
            ms.append(m.astype(NPBF16))
        _MASKS = ms
    return _MASKS


def _blockdiag(w):
    # (16, 64, 64) f32 -> (8, 128, 128) bf16 per-pair block diagonal
    o = np.zeros((NPAIR, 128, 128), np.float32)
    for p in range(NPAIR):
        o[p, :64, :64] = w[2 * p]
        o[p, 64:, 64:] = w[2 * p + 1]
    return o.astype(NPBF16)


def kernel(**inputs):
    global _NC_CACHE
    q = np.asarray(inputs["q"], np.float32)
    k = np.asarray(inputs["k"], np.float32)
    v = np.asarray(inputs["v"], np.float32)
    Wq = np.asarray(inputs["Wq"], np.float32)
    Wk = np.asarray(inputs["Wk"], np.float32)
    Wv = np.asarray(inputs["Wv"], np.float32)
    bq = np.asarray(inputs["bq"], np.float32)
    bk = np.asarray(inputs["bk"], np.float32)
    bv = np.asarray(inputs["bv"], np.float32)
    Wfc = np.asarray(inputs["Wfc"], np.float32)
    bfc = np.asarray(inputs["bfc"], np.float32)  # noqa: F841  cancels in LN
    gamma = np.asarray(inputs["gamma"], np.float32)
    beta = np.asarray(inputs["beta"], np.float32)

    if _NC_CACHE is None:
        _NC_CACHE = build()
    nc = _NC_CACHE
    masks = _masks()

    wq_h = np.ascontiguousarray(_blockdiag(Wq).transpose(1, 0, 2).reshape(128, -1))
    wk_h = np.ascontiguousarray(_blockdiag(Wk).transpose(1, 0, 2).reshape(128, -1))
    wv_h = np.ascontiguousarray(_blockdiag(Wv).transpose(1, 0, 2).reshape(128, -1))
    bq_h = np.ascontiguousarray(bq.reshape(NPAIR, 128).T)
    bk_h = np.ascontiguousarray(bk.reshape(NPAIR, 128).T)
    bv_h = np.ascontiguousarray(bv.reshape(NPAIR, 128).T)
    wfc_h = np.ascontiguousarray(Wfc.reshape(8, 128, E).transpose(1, 0, 2).reshape(128, -1))

    def _tile8(a):  # (S, E) -> transposed, pair-tiled (128, 8*S)
        t = a.T.reshape(NPAIR, 128, -1).transpose(1, 0, 2)
        return np.ascontiguousarray(t.reshape(128, -1))

    kts = [_tile8(k[b]).astype(NPBF16) for b in range(B)]
    vts = [_tile8(v[b]).astype(NPBF16) for b in range(B)]
    qts = [q[b].T for b in range(B)]

    in_maps = []
    for c in range(8):
        b, r = divmod(c, 4)
        in_maps.append(
            {
                "qt": np.ascontiguousarray(
                    qts[b][:, r::4].reshape(NPAIR, 128, SQ).transpose(1, 0, 2)
                    .reshape(128, -1)
                ).astype(NPBF16),
                "kt": kts[b],
                "vt": vts[b],
                "wq": wq_h,
                "wk": wk_h,
                "wv": wv_h,
                "bq": bq_h,
                "bk": bk_h,
                "bv": bv_h,
                "wfc": wfc_h,
                "vres": np.ascontiguousarray(v[b, r::4, :]).reshape(4, 128, E),
                "gamma": np.ascontiguousarray(gamma[r::4].reshape(4, 128).T),
                "beta": np.ascontiguousarray(beta[r::4].reshape(4, 128).T),
                "mask": masks[r],
            }
        )

    global _last_in_maps
    _last_in_maps = in_maps
    res = run_bass_kernel_spmd(nc, in_maps, list(range(8))).results
    full = np.empty((B, S, E), np.float32)
    for c in range(8):
        b, r = divmod(c, 4)
        full[b, r::4, :] = res[c]["out"].reshape(SQ, E)
    return full
